# revision 1
# baseline (speedup 1.0000x reference)
import numpy as np

# nn_ActionDecoder: LSTM decoder + per-scene GAT, teacher forcing ratio 1.
# Data-parallel over the agent axis across 8 NeuronCores (2048 agents each,
# scene groups of 8 never cross a shard boundary). Weights replicated.
#
# This implementation runs the sharded computation on the 8 neuron cores via
# jax (PJRT) with one pmap-style dispatch; all heavy math executes on-device.

PRED_LEN = 12
SEQ_LEN = 20
B = 16384
GROUP = 8
H = 128
IN = 64
F1 = 16
NH1 = 4
ALPHA = 0.2
NCORES = 8
BS = B // NCORES  # agents per core


def _build_jax_fn():
    import jax
    import jax.numpy as jnp

    def gat_layer(h, w, a_src, a_dst, bias):
        # h: [G, n, f_in] batched over groups; w: [k, f_in, f_out]
        hp = jnp.einsum("gnf,kfo->gkno", h, w)
        src = jnp.einsum("gkno,kop->gknp", hp, a_src)
        dst = jnp.einsum("gkno,kop->gknp", hp, a_dst)
        attn = src + jnp.swapaxes(dst, 2, 3)
        attn = jax.nn.leaky_relu(attn, negative_slope=ALPHA)
        attn = jax.nn.softmax(attn, axis=-1)
        return jnp.einsum("gknm,gkmo->gkno", attn, hp) + bias

    def shard_fn(action_real, h0, pred_goal, W_emb, b_emb, W_ih, W_hh, b_ih,
                 b_hh, W_goal, b_goal, w1, a_src1, a_dst1, bias1, w2, a_src2,
                 a_dst2, bias2, W_pos, b_pos):
        # action_real: [12, BS, 2]; h0: [BS, H]; pred_goal: [12, BS, 2]
        emb = action_real @ W_emb + b_emb
        c0 = jnp.zeros_like(h0)
        G_loc = BS // GROUP

        def step(carry, xs):
            h, c = carry
            x, goal = xs
            gates = x @ W_ih.T + b_ih + h @ W_hh.T + b_hh
            i, f, g, o = jnp.split(gates, 4, axis=-1)
            c = jax.nn.sigmoid(f) * c + jax.nn.sigmoid(i) * jnp.tanh(g)
            h = jax.nn.sigmoid(o) * jnp.tanh(c)
            h = h * jax.nn.softmax(goal @ W_goal + b_goal, axis=-1)
            hg = h.reshape(G_loc, GROUP, H)
            x1 = gat_layer(hg, w1, a_src1, a_dst1, bias1)          # [G,k,n,F1]
            x1 = jax.nn.elu(jnp.swapaxes(x1, 1, 2).reshape(G_loc, GROUP, NH1 * F1))
            x2 = gat_layer(x1, w2, a_src2, a_dst2, bias2)          # [G,1,n,H]
            h = x2[:, 0].reshape(BS, H)
            out = h @ W_pos + b_pos
            return (h, c), out

        (_, _), pred = jax.lax.scan(step, (h0, c0), (emb, pred_goal))
        return pred  # [12, BS, 2]

    return jax.pmap(shard_fn, axis_name="i",
                    in_axes=(0, 0, 0) + (None,) * 18)


_JAX_FN = None


def kernel(action_real, action_encoder_hidden_state, pred_goal, seq_start_end,
           teacher_forcing_ratio, W_emb, b_emb, W_ih, W_hh, b_ih, b_hh,
           W_goal, b_goal, w1, a_src1, a_dst1, bias1, w2, a_src2, a_dst2,
           bias2, W_pos, b_pos):
    global _JAX_FN
    import jax.numpy as jnp

    if _JAX_FN is None:
        _JAX_FN = _build_jax_fn()

    ar = np.asarray(action_real, np.float32)[-PRED_LEN:]          # [12, B, 2]
    h0 = np.asarray(action_encoder_hidden_state, np.float32)      # [B, H]
    pg = np.asarray(pred_goal, np.float32)                        # [12, B, 2]

    ar_s = ar.transpose(1, 0, 2).reshape(NCORES, BS, PRED_LEN, 2).transpose(0, 2, 1, 3)
    pg_s = pg.transpose(1, 0, 2).reshape(NCORES, BS, PRED_LEN, 2).transpose(0, 2, 1, 3)
    h0_s = h0.reshape(NCORES, BS, H)

    pred = _JAX_FN(
        jnp.asarray(ar_s), jnp.asarray(h0_s), jnp.asarray(pg_s),
        jnp.asarray(W_emb, jnp.float32), jnp.asarray(b_emb, jnp.float32),
        jnp.asarray(W_ih, jnp.float32), jnp.asarray(W_hh, jnp.float32),
        jnp.asarray(b_ih, jnp.float32), jnp.asarray(b_hh, jnp.float32),
        jnp.asarray(W_goal, jnp.float32), jnp.asarray(b_goal, jnp.float32),
        jnp.asarray(w1, jnp.float32), jnp.asarray(a_src1, jnp.float32),
        jnp.asarray(a_dst1, jnp.float32), jnp.asarray(bias1, jnp.float32),
        jnp.asarray(w2, jnp.float32), jnp.asarray(a_src2, jnp.float32),
        jnp.asarray(a_dst2, jnp.float32), jnp.asarray(bias2, jnp.float32),
        jnp.asarray(W_pos, jnp.float32), jnp.asarray(b_pos, jnp.float32),
    )
    pred = np.asarray(pred)                     # [8, 12, BS, 2]
    out = pred.transpose(1, 0, 2, 3).reshape(PRED_LEN, B, 2)
    return out.astype(np.float32)
